# revision 25
# baseline (speedup 1.0000x reference)
"""Trainium2 Bass kernel for CrossTokenMLPAggregator (top-k masked attention aggregation).

Computes, for full inputs
    mlp_hidden   [B=2, T=2048, H=1024] f32
    attn_weights [B=2, Hh=16, T=2048, T=2048] f32
the reference:
    W = attn_weights.mean(axis=1)              # [B, T, T]
    keep top-8 per query row, renormalize kept mass to sum 1
    out = einsum('bts,bsh->bth', W_sparse, mlp_hidden)

Sharding: 8 cores, each owns 512 query rows (core c -> batch c//4,
query rows (c%4)*512 ...). Each core streams its [16, 512, 2048] slice of
attn_weights (the 512 MiB input dominates; split is exact, no duplication).

Per 128-row query tile the 16 head slices stream in via the Sync-engine
DMA queues at ~2.3us/head and are summed in f32: 12 adds on DVE, 3 on
GpSimd (heads 1,4,7,10 into a partial), sized so each engine keeps pace
with the arrival rate under SBUF port contention.  Top-8 per row via the
DVE max8 instruction; the mask (W >= v8)*W runs as one DVE
scalar_tensor_tensor emitting bf16; masked rows are transposed on the
TensorEngine (bf16) and contracted with a bf16 copy of mlp_hidden (staged
through the heads pool, converted once on ScalarE).  1/sum(top8) comes
from the max8 output (ScalarE accumulate) and is folded into the
PSUM->SBUF eviction; output stores issue from ScalarE so the Sync
engine's in-order input-DMA stream never blocks on compute.

The last tile's epilogue is latency-critical (nothing overlaps it), so
there the final head-add and partial-combine are split into four 512-col
chunks behind a 4-way split of the last head's DMA, max8 runs per half and
is merged, the mask runs as four chunked GpSimd STTs feeding the four
transpose groups as they become ready, and a chain of throwaway acc-chunk
transposes keyed on the column combines keeps the PE clock ramped (cold
PE runs at 0.65-1.2 GHz vs 2.4 GHz warm) before the real burst.
"""

import numpy as np

B, T, H, Hh, K = 2, 2048, 1024, 16, 8
NCORES = 8
QPC = (B * T) // NCORES          # 512 query rows per core
P = 128                          # partitions
TQ_TILES = QPC // P              # 4 tiles of 128 query rows
S_CHUNKS = T // P                # 16 contraction chunks
EPS_SUM = np.float32(1e-8) * np.float32(16.0)  # EPS in head-sum domain
POOL_HEADS = (1, 4, 7, 10)       # heads summed on GpSimd instead of DVE
GADGETS = True                   # last-tile tail gadgets (A/B flag)

_compiled = {}


def _build_nc():
    import concourse.bass as bass
    import concourse.bacc as bacc
    import concourse.mybir as mybir
    import concourse.tile as tile
    from concourse import masks

    f32 = mybir.dt.float32
    bf16 = mybir.dt.bfloat16
    COPY = mybir.ActivationFunctionType.Copy
    nc = bacc.Bacc(
        "TRN2",
        target_bir_lowering=False,
        debug=False,
        enable_asserts=False,
        num_devices=NCORES,
    )
    attn = nc.dram_tensor("attn", [Hh, QPC, T], f32, kind="ExternalInput").ap()
    mlp = nc.dram_tensor("mlp", [T, H], f32, kind="ExternalInput").ap()
    out = nc.dram_tensor("out", [QPC, H], f32, kind="ExternalOutput").ap()

    with tile.TileContext(nc) as tc:
        with (
            tc.tile_pool(name="persist", bufs=1) as persist,
            tc.tile_pool(name="heads", bufs=10) as heads,
            tc.tile_pool(name="acc", bufs=2) as accp,
            tc.tile_pool(name="ppool", bufs=2) as ppp,
            tc.tile_pool(name="wm", bufs=2) as wmp,
            tc.tile_pool(name="wmt", bufs=2) as wmtp,
            tc.tile_pool(name="small", bufs=2) as small,
            tc.tile_pool(name="outsb", bufs=2) as outsbp,
            tc.tile_pool(name="tp_psum", bufs=3, space="PSUM") as tp_psum,
            tc.tile_pool(name="warm_psum", bufs=1, space="PSUM") as warm_psum,
            tc.tile_pool(name="mm_psum", bufs=2, space="PSUM") as mm_psum,
        ):
            # mlp_hidden -> SBUF as bf16, staged in f32 through the heads
            # pool (2 chunks per staging tile) and converted on ScalarE.
            # The staging DMAs issue from the (otherwise idle) GpSimd engine
            # so the Sync engine streams tile 0's attention heads from t=0 --
            # both DGE rings fill the DMA queues in parallel at startup.
            mlp_sb = persist.tile([P, S_CHUNKS, H], bf16)
            for g in range(S_CHUNKS // 2):
                ms = heads.tile([P, T], f32, tag="ht")
                for j in range(2):
                    c = 2 * g + j
                    nc.gpsimd.dma_start(
                        out=ms[:, j * H : (j + 1) * H],
                        in_=mlp[c * P : (c + 1) * P, :],
                    )
                nc.scalar.copy(mlp_sb[:, 2 * g : 2 * g + 2, :], ms)
            ident = persist.tile([P, P], bf16)
            masks.make_identity(nc, ident[:])
            identf = persist.tile([P, P], f32)
            masks.make_identity(nc, identf[:])

            for t in range(TQ_TILES):
                q = slice(t * P, (t + 1) * P)
                last = (t == TQ_TILES - 1) and GADGETS
                # ---- head-sum accumulation in f32 (selection-exact) ----
                acc = accp.tile([P, T], f32)
                pp = ppp.tile([P, T], f32, tag="pp")
                nc.sync.dma_start(out=acc, in_=attn[0, q, :])
                hts = {}
                for h in range(1, Hh - 1):
                    ht = heads.tile([P, T], f32, tag="ht")
                    nc.sync.dma_start(out=ht, in_=attn[h, q, :])
                    if h == POOL_HEADS[0]:
                        hts[h] = ht
                    elif h == POOL_HEADS[1]:
                        nc.gpsimd.tensor_add(out=pp, in0=hts.pop(POOL_HEADS[0]), in1=ht)
                    elif h in POOL_HEADS:
                        nc.gpsimd.tensor_add(out=pp, in0=pp, in1=ht)
                    else:
                        nc.vector.tensor_add(out=acc, in0=acc, in1=ht)
                # final head + partial-combine; column-chunked on the last
                # tile so the epilogue starts before the full row lands.
                # Mid-tile combines run on GpSimd to keep DVE within its
                # per-tile budget; the last tile keeps them on DVE (latency).
                hl = heads.tile([P, T], f32, tag="ht")
                if last:
                    for j in range(4):
                        sl = slice(j * (T // 4), (j + 1) * (T // 4))
                        nc.sync.dma_start(out=hl[:, sl], in_=attn[Hh - 1, q, sl])
                        nc.vector.tensor_add(
                            out=acc[:, sl], in0=acc[:, sl], in1=hl[:, sl]
                        )
                        nc.vector.tensor_add(
                            out=acc[:, sl], in0=acc[:, sl], in1=pp[:, sl]
                        )
                else:
                    nc.sync.dma_start(out=hl, in_=attn[Hh - 1, q, :])
                    nc.vector.tensor_add(out=acc, in0=acc, in1=hl)
                    nc.vector.tensor_add(out=acc, in0=acc, in1=pp)

                # ---- top-8 values per row ----
                g8 = small.tile([P, K], f32, tag="g8")
                if last:
                    # halves + merge; PE warm-up transposes keyed on the
                    # column combines keep the TensorEngine clock ramped
                    wrm = warm_psum.tile([P, P], f32, tag="wrm")
                    for c in range(10):
                        nc.tensor.transpose(
                            wrm[:], acc[:, (c % 8) * P : (c % 8 + 1) * P],
                            identf[:],
                        )
                    mh = small.tile([P, 2 * K], f32, tag="mh")
                    nc.vector.max(out=mh[:, :K], in_=acc[:, : T // 2])
                    nc.vector.max(out=mh[:, K:], in_=acc[:, T // 2 :])
                    nc.vector.max(out=g8, in_=mh)
                else:
                    nc.vector.max(out=g8, in_=acc)

                # kept mass = sum of the top-8 values (ScalarE accumulate),
                # clipped for parity with the reference, then reciprocal.
                mxc = small.tile([P, K], f32, tag="mxc")
                ssum = small.tile([P, 1], f32, tag="ssum")
                nc.scalar.activation(out=mxc, in_=g8, func=COPY, accum_out=ssum)
                nc.vector.tensor_scalar_max(ssum, ssum, float(EPS_SUM))
                rcp = small.tile([P, 1], f32, tag="rcp")
                nc.vector.reciprocal(rcp, ssum)

                # ---- mask: wm = (acc >= v8) * acc -> bf16;
                # chunked on the last tile to feed transpose groups early ----
                wm = wmp.tile([P, T], bf16, tag="wm")
                NMSK = 4 if last else 1
                WM = T // NMSK
                for j in range(NMSK):
                    sl = slice(j * WM, (j + 1) * WM)
                    nc.vector.scalar_tensor_tensor(
                        out=wm[:, sl],
                        in0=acc[:, sl],
                        scalar=g8[:, K - 1 : K],
                        in1=acc[:, sl],
                        op0=mybir.AluOpType.is_ge,
                        op1=mybir.AluOpType.mult,
                    )

                # ---- transpose + matmul (bf16), pipelined per 4-chunk group ----
                wmt = wmtp.tile([P, S_CHUNKS, P], bf16, tag="wmt")
                acc_ps = mm_psum.tile([P, H], f32, tag="acc_ps")
                for g in range(S_CHUNKS // 4):
                    pt = tp_psum.tile([P, 4 * P], bf16, tag="pt")
                    for j in range(4):
                        c = 4 * g + j
                        nc.tensor.transpose(
                            pt[:, j * P : (j + 1) * P],
                            wm[:, c * P : (c + 1) * P],
                            ident[:],
                        )
                    nc.scalar.copy(wmt[:, 4 * g : 4 * g + 4, :], pt[:])
                    for j in range(4):
                        c = 4 * g + j
                        for nh in range(H // 512):
                            nsl = slice(nh * 512, (nh + 1) * 512)
                            nc.tensor.matmul(
                                acc_ps[:, nsl],
                                lhsT=wmt[:, c, :],
                                rhs=mlp_sb[:, c, nsl],
                                start=(c == 0),
                                stop=(c == S_CHUNKS - 1),
                                skip_group_check=True,
                            )

                # ---- renormalize + evict on ScalarE, store from ScalarE so
                # the Sync engine's input-DMA stream is never blocked ----
                osb = outsbp.tile([P, H], f32, tag="osb")
                nc.scalar.activation(out=osb, in_=acc_ps, func=COPY, scale=rcp[:, :])
                nc.scalar.dma_start(out=out[q, :], in_=osb)

    nc.compile()
    return nc


def _get_nc():
    if "nc" not in _compiled:
        _compiled["nc"] = _build_nc()
    return _compiled["nc"]


def kernel(mlp_hidden: np.ndarray, attn_weights: np.ndarray) -> np.ndarray:
    from concourse.bass_utils import run_bass_kernel_spmd

    mlp_hidden = np.ascontiguousarray(mlp_hidden, dtype=np.float32)
    attn_weights = np.ascontiguousarray(attn_weights, dtype=np.float32)
    assert mlp_hidden.shape == (B, T, H)
    assert attn_weights.shape == (B, Hh, T, T)

    nc = _get_nc()
    in_maps = []
    for c in range(NCORES):
        b = c // (NCORES // B)
        q0 = (c % (NCORES // B)) * QPC
        in_maps.append(
            {
                "attn": np.ascontiguousarray(attn_weights[b, :, q0 : q0 + QPC, :]),
                "mlp": mlp_hidden[b],
            }
        )
    res = run_bass_kernel_spmd(nc, in_maps, list(range(NCORES)))
    out = np.empty((B, T, H), dtype=np.float32)
    for c in range(NCORES):
        b = c // (NCORES // B)
        q0 = (c % (NCORES // B)) * QPC
        out[b, q0 : q0 + QPC] = res.results[c]["out"]
    return out
